# revision 4
# baseline (speedup 1.0000x reference)
"""Trainium2 Bass kernel for a post-LN transformer encoder block (full IO).

Sharding: 8-way data parallel over (batch, sequence-half): core c handles
batch c//2 and query rows [(c%2)*1024, (c%2)*1024+1024). Each core computes
K/V for its whole batch (duplicated across the 2 cores sharing a batch), so
there are no cross-core collectives. For odd cores the host rotates the key
axis by 1024 so every core's queries sit at columns [0, 1024) — one NEFF
serves all 8 cores.

On-chip layout is feature-major ([d, seq]) so the TensorEngine — which
contracts over the partition dim — chains projections without on-chip
transposes; the host transposes x on the way in and the output back.

Softmax: scoresT keeps keys on the partition axis, so exp() comes off the
scalar engine with the padding-mask bias folded in per-partition, and the
denominator falls out of the ctx matmul for free via a ones column
interleaved into V. Row broadcasts (1/den, LN scale/shift) go through a
K=1 PE matmul (bf16) and a DRAM-bounce broadcast DMA (fp32) respectively.
"""

import sys
import numpy as np

for _p in ("/root/.axon_site", "/root/.axon_site/_ro/trn_rl_repo",
           "/root/.axon_site/_ro/pypackages", "/opt/trn_rl_repo"):
    if _p not in sys.path:
        sys.path.append(_p)

import ml_dtypes

B, S, D, H, DFF = 4, 2048, 1024, 16, 4096
DK = D // H            # 64
EPS = 1e-9
N_CORES = 8
M = S // 2             # queries per core
NB = ml_dtypes.bfloat16

KI = D // 128          # 8 contraction tiles over the model dim
KT = S // 128          # 16 key tiles
QC = M // 512          # 2 query chunks
VST = DK + 1           # 65: V head block incl. ones column


def build(mask_has_zeros: bool):
    import concourse.bass as bass
    import concourse.mybir as mybir
    import concourse.tile as tile
    from concourse import bacc
    import contextlib

    BF = mybir.dt.bfloat16
    F32 = mybir.dt.float32
    PF32 = mybir.dt.float32
    ACTF = mybir.ActivationFunctionType
    AL = mybir.AluOpType

    nc = bacc.Bacc("TRN2", target_bir_lowering=False, debug=False,
                   num_devices=N_CORES)

    xtb_d = nc.dram_tensor("xtb", [D, S], BF, kind="ExternalInput").ap()
    xq32_d = nc.dram_tensor("xq32", [D, M], F32, kind="ExternalInput").ap()
    wq_d = nc.dram_tensor("wqb", [D, D], BF, kind="ExternalInput").ap()
    wk_d = nc.dram_tensor("wkb", [D, D], BF, kind="ExternalInput").ap()
    wv_d = nc.dram_tensor("wvb", [D, D], BF, kind="ExternalInput").ap()
    wo_d = nc.dram_tensor("wob", [D, D], BF, kind="ExternalInput").ap()
    w1_d = nc.dram_tensor("w1b", [D, DFF], BF, kind="ExternalInput").ap()
    w2_d = nc.dram_tensor("w2b", [DFF, D], BF, kind="ExternalInput").ap()
    mb_d = nc.dram_tensor("maskb", [128, KT], F32, kind="ExternalInput").ap()
    # cvec per LN i: [g, -g/D, g*EPS+bt]
    cv_d = nc.dram_tensor("cvec", [1, 8], F32, kind="ExternalInput").ap()
    out_d = nc.dram_tensor("outT", [D, M], F32, kind="ExternalOutput").ap()
    scratch_d = nc.dram_tensor("lnrows", [2, M], F32).ap()

    with tile.TileContext(nc) as tc:
        with contextlib.ExitStack() as ctx:
            big = ctx.enter_context(tc.tile_pool(name="big", bufs=1))
            mid = ctx.enter_context(tc.tile_pool(name="mid", bufs=1))
            ps = ctx.enter_context(tc.tile_pool(name="ps", bufs=1, space="PSUM"))

            # slot families (per-partition bytes x bufs):
            #  big4: 4KB x16  — xtb(P), kt(P->A), trunk(R->L)
            #  vt:   2080B x16 — V (P->A)
            #  xq/rows/ab: f32 helpers
            #  b2k:  2KB x17  — qt, ctxt, ln casts, x2b
            #  wst:  2KB x16  — streamed weight tiles
            #  scr:  2KB x8   — expT(A), bcs/den(A), ffb(F)
            def big4(name):
                return big.tile([128, 2048], BF, tag="big4", bufs=16, name=name)

            def big4f(name):
                return big.tile([128, 1024], F32, tag="big4", bufs=16, name=name)

            def b2k(name):
                return mid.tile([128, 1024], BF, tag="b2k", bufs=17, name=name)

            def wst(name):
                return mid.tile([128, 1024], BF, tag="wst", bufs=16, name=name)

            def scr(shape, dt, name):
                return mid.tile(shape, dt, tag="scr", bufs=8, name=name)

            def accp(name):
                return ps.tile([128, 512], PF32, tag="acc", bufs=3, name=name)

            # ---------- constants ----------
            ones_row = mid.tile([1, 64], BF, tag="ones_r", bufs=1)
            nc.vector.memset(ones_row, 1.0)
            ones_col = mid.tile([128, 1], BF, tag="ones_c", bufs=1)
            nc.vector.memset(ones_col, 1.0)
            cvec = mid.tile([1, 8], F32, tag="cvec", bufs=1)
            nc.sync.dma_start(out=cvec, in_=cv_d)
            if mask_has_zeros:
                mbt = mid.tile([128, KT], F32, tag="mbt", bufs=1)
                nc.sync.dma_start(out=mbt, in_=mb_d)

            # ---------- load xT (bf16, full batch) ----------
            xtb = []
            for ki in range(KI):
                t = big4(f"xtb{ki}")
                nc.sync.dma_start(out=t, in_=xtb_d[ki * 128:(ki + 1) * 128, :])
                xtb.append(t)

            # ---------- projections (feature-major outputs) ----------
            def proj(w_dram, n_cols, out_tiles, name):
                wts = []
                for ki in range(KI):
                    wt = wst(f"{name}w{ki}")
                    nc.sync.dma_start(out=wt, in_=w_dram[ki * 128:(ki + 1) * 128, :])
                    wts.append(wt)
                for mo in range(KI):
                    for c in range(n_cols // 512):
                        acc = accp(f"{name}ps{mo}_{c}")
                        for ki in range(KI):
                            nc.tensor.matmul(
                                acc[:, :],
                                wts[ki][:, mo * 128:(mo + 1) * 128],
                                xtb[ki][:, c * 512:(c + 1) * 512],
                                start=(ki == 0), stop=(ki == KI - 1),
                            )
                        nc.vector.tensor_copy(
                            out_tiles[mo][:, c * 512:(c + 1) * 512], acc[:, :])

            qt = [b2k(f"qt{i}") for i in range(KI)]
            kt = [big4(f"kt{i}") for i in range(KI)]
            proj(wq_d, M, qt, "q")
            proj(wk_d, S, kt, "k")

            # ---------- V (sequence-major, head-interleaved + ones col) ----------
            wvts = []
            for ki in range(KI):
                wt = wst(f"vw{ki}")
                nc.sync.dma_start(out=wt, in_=wv_d[ki * 128:(ki + 1) * 128, :])
                wvts.append(wt)
            vt = []
            for k in range(KT):
                v = big.tile([128, H * VST], BF, tag="vt", bufs=KT, name=f"vt{k}")
                vt.append(v)
                for c in range(2):
                    acc = accp(f"vps{k}_{c}")
                    for ki in range(KI):
                        nc.tensor.matmul(
                            acc[:, :],
                            xtb[ki][:, k * 128:(k + 1) * 128],
                            wvts[ki][:, c * 512:(c + 1) * 512],
                            start=(ki == 0), stop=(ki == KI - 1),
                        )
                    dst = v[:, c * 8 * VST:(c * 8 + 8) * VST].rearrange(
                        "p (h j) -> p h j", j=VST)[:, :, 0:DK]
                    src = acc[:, :].rearrange("p (h j) -> p h j", j=DK)
                    nc.vector.tensor_copy(dst, src)
                ones_view = v[:, :].rearrange("p (h j) -> p h j", j=VST)[:, :, DK:DK + 1]
                nc.vector.memset(ones_view, 1.0)

            # wo streamed early; held until the attn-out matmuls
            wots = []
            for ki in range(KI):
                wt = wst(f"ow{ki}")
                nc.sync.dma_start(out=wt, in_=wo_d[ki * 128:(ki + 1) * 128, :])
                wots.append(wt)

            # ---------- attention ----------
            ctxt = [b2k(f"ctxt{i}") for i in range(KI)]
            for h in range(H):
                ht = h // 2
                hb = (h % 2) * 64
                cps = [ps.tile([65, 512], PF32, tag="ctxp", bufs=2,
                               name=f"cps{h}_{c}") for c in range(QC)]
                for k in range(KT):
                    et = scr([128, 1024], BF, f"et{h}_{k}")
                    for c in range(QC):
                        sps = ps.tile([128, 512], PF32, tag="sc", bufs=2,
                                      name=f"sps{h}_{k}_{c}")
                        nc.tensor.matmul(
                            sps[:, :],
                            kt[ht][hb:hb + 64, k * 128:(k + 1) * 128],
                            qt[ht][hb:hb + 64, c * 512:(c + 1) * 512],
                            start=True, stop=True,
                        )
                        nc.scalar.activation(
                            out=et[:, c * 512:(c + 1) * 512], in_=sps[:, :],
                            func=ACTF.Exp,
                            bias=(mbt[:, k:k + 1] if mask_has_zeros else 0.0),
                            scale=0.125,
                        )
                    for c in range(QC):
                        nc.tensor.matmul(
                            cps[c][:, :],
                            vt[k][:, h * VST:(h + 1) * VST],
                            et[:, c * 512:(c + 1) * 512],
                            start=(k == 0), stop=(k == KT - 1),
                        )
                for c in range(QC):
                    den = scr([1, 512], BF, f"den{h}_{c}")
                    with nc.allow_low_precision(reason="bf16 softmax denom"):
                        nc.vector.reciprocal(out=den[:, :], in_=cps[c][64:65, :])
                    bps = ps.tile([64, 512], PF32, tag="bc", bufs=1,
                                  name=f"bps{h}_{c}")
                    nc.tensor.matmul(bps[:, :], ones_row[:, :], den[:, :],
                                     start=True, stop=True)
                    bsb = scr([64, 512], F32, f"bsb{h}_{c}")
                    nc.vector.tensor_copy(bsb[:, :], bps[:, :])
                    with nc.allow_low_precision(reason="ctx stored bf16"):
                        nc.vector.tensor_mul(
                            ctxt[ht][hb:hb + 64, c * 512:(c + 1) * 512],
                            cps[c][0:64, :], bsb[:, :])

            # ---------- attn out + residual -> fp32 trunk ----------
            trunk = []
            for mo in range(KI):
                xq = big.tile([128, 1024], F32, tag="xq", bufs=2, name=f"xq{mo}")
                nc.sync.dma_start(out=xq, in_=xq32_d[mo * 128:(mo + 1) * 128, :])
                tr = big4f(f"trunk{mo}")
                trunk.append(tr)
                for c in range(QC):
                    acc = accp(f"aops{mo}_{c}")
                    for ki in range(KI):
                        nc.tensor.matmul(
                            acc[:, :],
                            wots[ki][:, mo * 128:(mo + 1) * 128],
                            ctxt[ki][:, c * 512:(c + 1) * 512],
                            start=(ki == 0), stop=(ki == KI - 1),
                        )
                    nc.vector.tensor_add(tr[:, c * 512:(c + 1) * 512],
                                         acc[:, :], xq[:, c * 512:(c + 1) * 512])

            # ---------- feature-major layernorm, applied in place ----------
            def layer_norm(g_idx, nm):
                xb = [b2k(f"{nm}xb{i}") for i in range(KI)]
                for mo in range(KI):
                    with nc.allow_low_precision(reason="ln stats in bf16"):
                        nc.vector.tensor_copy(xb[mo][:, :], trunk[mo][:, :])
                r_s1 = big.tile([1, 1024], F32, tag="rows", bufs=3,
                                name=f"{nm}rs1")
                r_tmp = big.tile([1, 1024], F32, tag="rows", bufs=3,
                                 name=f"{nm}rtmp")
                r_istd = big.tile([1, 1024], F32, tag="rows", bufs=3,
                                  name=f"{nm}ristd")
                s1 = [ps.tile([1, 512], PF32, tag="acc", bufs=3,
                              name=f"{nm}s1_{c}") for c in range(QC)]
                for mo in range(KI):
                    for c in range(QC):
                        nc.tensor.matmul(s1[c][:, :], ones_col[:, :],
                                         xb[mo][:, c * 512:(c + 1) * 512],
                                         start=(mo == 0), stop=(mo == KI - 1))
                for c in range(QC):
                    cs = slice(c * 512, (c + 1) * 512)
                    nc.vector.tensor_copy(r_s1[:, cs], s1[c][:, :])
                s2 = [ps.tile([1, 512], PF32, tag="acc", bufs=3,
                              name=f"{nm}s2_{c}") for c in range(QC)]
                for mo in range(KI):
                    sq = scr([128, 1024], BF, f"{nm}sq{mo}")
                    nc.scalar.activation(out=sq[:, :], in_=xb[mo][:, :],
                                         func=ACTF.Square)
                    for c in range(QC):
                        nc.tensor.matmul(s2[c][:, :], ones_col[:, :],
                                         sq[:, c * 512:(c + 1) * 512],
                                         start=(mo == 0), stop=(mo == KI - 1))
                for c in range(QC):
                    cs = slice(c * 512, (c + 1) * 512)
                    nc.scalar.activation(out=r_tmp[:, cs], in_=r_s1[:, cs],
                                         func=ACTF.Square)
                    nc.vector.scalar_tensor_tensor(
                        out=r_tmp[:, cs], in0=r_tmp[:, cs],
                        scalar=-1.0 / D, in1=s2[c][:, :],
                        op0=AL.mult, op1=AL.add)
                    nc.scalar.activation(out=r_tmp[:, cs], in_=r_tmp[:, cs],
                                         func=ACTF.Sqrt, scale=1.0 / (D - 1))
                    nc.vector.reciprocal(out=r_istd[:, cs], in_=r_tmp[:, cs])
                    nc.vector.tensor_mul(r_tmp[:, cs], r_s1[:, cs],
                                         r_istd[:, cs])
                    nc.vector.tensor_scalar(
                        out=r_tmp[:, cs], in0=r_tmp[:, cs],
                        scalar1=cvec[:, 3 * g_idx + 1:3 * g_idx + 2],
                        scalar2=cvec[:, 3 * g_idx + 2:3 * g_idx + 3],
                        op0=AL.mult, op1=AL.add)
                    nc.vector.tensor_scalar_mul(
                        out=r_istd[:, cs], in0=r_istd[:, cs],
                        scalar1=cvec[:, 3 * g_idx:3 * g_idx + 1])
                nc.sync.dma_start(out=scratch_d[0:1, :], in_=r_istd[:, :])
                nc.sync.dma_start(out=scratch_d[1:2, :], in_=r_tmp[:, :])
                ab = big.tile([128, 2, 1024], F32, tag="ab", bufs=1, name=f"{nm}ab")
                bsrc = bass.AP(tensor=scratch_d.tensor, offset=0,
                               ap=[[0, 128], [M, 2], [1, M]])
                nc.sync.dma_start(out=ab[:, :, :], in_=bsrc)
                for mo in range(KI):
                    nc.vector.tensor_mul(trunk[mo][:, :], trunk[mo][:, :],
                                         ab[:, 0, :])
                    nc.vector.tensor_add(trunk[mo][:, :], trunk[mo][:, :],
                                         ab[:, 1, :])

            layer_norm(0, "ln1")

            # ---------- FFN ----------
            x2b = [b2k(f"x2b{i}") for i in range(KI)]
            for mo in range(KI):
                with nc.allow_low_precision(reason="ffn input bf16"):
                    nc.vector.tensor_copy(x2b[mo][:, :], trunk[mo][:, :])

            for g in range(4):
                w1ts = []
                for ki in range(KI):
                    wt = wst(f"w1t{g}_{ki}")
                    nc.sync.dma_start(
                        out=wt, in_=w1_d[ki * 128:(ki + 1) * 128,
                                         g * 1024:(g + 1) * 1024])
                    w1ts.append(wt)
                ffb = []
                for fl in range(8):
                    fb = scr([128, 1024], BF, f"ffb{g}_{fl}")
                    ffb.append(fb)
                    for c in range(QC):
                        acc = accp(f"f1ps{g}_{fl}_{c}")
                        for ki in range(KI):
                            nc.tensor.matmul(
                                acc[:, :],
                                w1ts[ki][:, fl * 128:(fl + 1) * 128],
                                x2b[ki][:, c * 512:(c + 1) * 512],
                                start=(ki == 0), stop=(ki == KI - 1),
                            )
                        nc.scalar.activation(out=fb[:, c * 512:(c + 1) * 512],
                                             in_=acc[:, :], func=ACTF.Relu)
                w2ts = []
                for fl in range(8):
                    wt = wst(f"w2t{g}_{fl}")
                    nc.sync.dma_start(
                        out=wt,
                        in_=w2_d[(g * 8 + fl) * 128:(g * 8 + fl + 1) * 128, :])
                    w2ts.append(wt)
                for mo in range(KI):
                    for c in range(QC):
                        acc = accp(f"f2ps{g}_{mo}_{c}")
                        for fl in range(8):
                            nc.tensor.matmul(
                                acc[:, :],
                                w2ts[fl][:, mo * 128:(mo + 1) * 128],
                                ffb[fl][:, c * 512:(c + 1) * 512],
                                start=(fl == 0), stop=(fl == 7),
                            )
                        cs = slice(c * 512, (c + 1) * 512)
                        nc.vector.tensor_add(trunk[mo][:, cs], trunk[mo][:, cs],
                                             acc[:, :])

            layer_norm(1, "ln2")

            for mo in range(KI):
                nc.sync.dma_start(out=out_d[mo * 128:(mo + 1) * 128, :],
                                  in_=trunk[mo][:, :])

    nc.compile()
    return nc


_NC_CACHE = {}


def _get_nc(mask_has_zeros: bool):
    if mask_has_zeros not in _NC_CACHE:
        _NC_CACHE[mask_has_zeros] = build(mask_has_zeros)
    return _NC_CACHE[mask_has_zeros]


def _reference_numpy(x, mask, wq, bq, wk, bk, wv, bv, wo, bo, w1, b1, w2, b2,
                     g1, bt1, g2, bt2):
    import math
    out = np.zeros_like(x)

    def ln(v, g, bt):
        mean = v.mean(-1, keepdims=True)
        std = v.std(-1, keepdims=True, ddof=1)
        return g * ((v - mean) / std + EPS) + bt

    for b in range(B):
        xb = x[b]
        q = (xb @ wq + bq).reshape(S, H, DK).transpose(1, 0, 2)
        k = (xb @ wk + bk).reshape(S, H, DK).transpose(1, 0, 2)
        v = (xb @ wv + bv).reshape(S, H, DK).transpose(1, 0, 2)
        ctx = np.zeros((H, S, DK), np.float32)
        mrow = mask[b, 0, 0, :]
        for h in range(H):
            sc = (q[h] @ k[h].T) / math.sqrt(DK)
            sc = np.where(mrow[None, :] == 0, np.float32(-1e9), sc)
            e = np.exp(sc - sc.max(-1, keepdims=True))
            p = e / e.sum(-1, keepdims=True)
            ctx[h] = p @ v[h]
        cx = ctx.transpose(1, 0, 2).reshape(S, D)
        x1 = ln(xb + cx @ wo + bo, g1, bt1)
        ff = np.maximum(x1 @ w1 + b1, 0.0) @ w2 + b2
        out[b] = ln(x1 + ff, g2, bt2)
    return out


def kernel(**inputs) -> np.ndarray:
    from concourse.bass_utils import run_bass_kernel_spmd

    x = np.asarray(inputs["x"], np.float32)
    mask = np.asarray(inputs["mask"])
    wq, wk, wv, wo = (np.asarray(inputs[k], np.float32)
                      for k in ("wq", "wk", "wv", "wo"))
    w1 = np.asarray(inputs["w1"], np.float32)
    w2 = np.asarray(inputs["w2"], np.float32)
    g1 = float(np.asarray(inputs["g1"]))
    bt1 = float(np.asarray(inputs["bt1"]))
    g2 = float(np.asarray(inputs["g2"]))
    bt2 = float(np.asarray(inputs["bt2"]))
    biases = [np.asarray(inputs[k], np.float32)
              for k in ("bq", "bk", "bv", "bo", "b1", "b2")]

    if any(np.abs(b).max() > 0 for b in biases):
        # biases are identically zero for this module's init; exact but slow
        # host fallback keeps the kernel fully general.
        return _reference_numpy(
            x, mask, wq, biases[0], wk, biases[1], wv, biases[2], wo,
            biases[3], w1, biases[4], w2, biases[5], g1, bt1, g2, bt2)

    mask_has_zeros = bool((mask == 0).any())
    nc = _get_nc(mask_has_zeros)
    in_maps = _prepare_in_maps(x, mask, wq, wk, wv, wo, w1, w2,
                               g1, bt1, g2, bt2)

    res = run_bass_kernel_spmd(nc, in_maps, core_ids=list(range(N_CORES)))
    globals()["LAST_RESULTS"] = res

    out = np.empty((B, S, D), np.float32)
    for core in range(N_CORES):
        b = core // 2
        qoff = (core % 2) * M
        out[b, qoff:qoff + M, :] = res.results[core]["outT"].T
    return out


def _prepare_in_maps(x, mask, wq, wk, wv, wo, w1, w2, g1, bt1, g2, bt2):
    wqb, wkb, wvb, wob = (w.astype(NB) for w in (wq, wk, wv, wo))
    w1b = w1.astype(NB)
    w2b = w2.astype(NB)
    cvec = np.array([[g1, -g1 / D, g1 * EPS + bt1,
                      g2, -g2 / D, g2 * EPS + bt2, 0.0, 0.0]], np.float32)

    in_maps = []
    for core in range(N_CORES):
        b = core // 2
        qoff = (core % 2) * M
        xT = np.ascontiguousarray(x[b].T)
        mrow = np.where(mask[b, 0, 0, :] == 0, np.float32(-1e9),
                        np.float32(0.0)).astype(np.float32)
        if qoff:
            # rotate keys so this core's queries sit at columns [0, M)
            xT_k = np.concatenate([xT[:, M:], xT[:, :M]], axis=1)
            mrow = np.concatenate([mrow[M:], mrow[:M]])
        else:
            xT_k = xT
        in_maps.append({
            "xtb": xT_k.astype(NB),
            "xq32": np.ascontiguousarray(xT[:, qoff:qoff + M]),
            "wqb": wqb, "wkb": wkb, "wvb": wvb, "wob": wob,
            "w1b": w1b, "w2b": w2b,
            "maskb": np.ascontiguousarray(mrow.reshape(KT, 128).T),
            "cvec": cvec,
        })
    return in_maps


if __name__ == "__main__":
    d = np.load("/root/problem/ref_cache.npz")
    inputs = {k: d[k] for k in d.files if k != "exp"}
    got = kernel(**inputs)
    exp = d["exp"]
    err = np.abs(got - exp)
    print("max abs err:", err.max())
    print("rel max:", err.max() / np.abs(exp).max())
    print("rel l2:", np.linalg.norm(err) / np.linalg.norm(exp))

